# revision 8
# baseline (speedup 1.0000x reference)
"""Trainium2 Bass kernel for DEMA (Holt's linear trend) decomposition.

reference:  ma = DEMA(x) along time (alpha=0.3, beta=0.1), res = x - ma,
            x: [32, 4096, 128] fp32, returns (res, ma).

Approach: the DEMA is a 2x2 linear recurrence v_t = A v_{t-1} + c x_t with
spectral radius sqrt(0.7) ~ 0.837, so the impulse response decays below 1e-10
within 128 steps.  The scan therefore collapses to a banded lower-triangular
matmul (FIR) over time:  with 128-step time blocks,
    ma_blk[i] = W0 @ x_blk[i] + W1 @ x_blk[i-1]
with constant 128x128 Toeplitz coefficient blocks (W0 lower-triangular band,
W1 the band crossing the block boundary).  Blocks 0/1 get exact special
matrices carrying the s0/b0 initial-condition terms.  This maps onto the
TensorEngine: contraction over source-time (partitions), (batch x channel)
on the moving free dim.

Sharding: batch 32 -> 4 per core across 8 cores, no communication.
"""

import numpy as np

ALPHA = 0.3
BETA = 0.1
P = 128          # time block = partition dim
B, T, C = 32, 4096, 128
NCORES = 8
BL = B // NCORES  # local batch = 4
NB = T // P       # 32 time blocks
G = 8             # time blocks per DMA group
NG = NB // G      # groups
FREE = BL * C     # matmul moving free dim = 512 (fp32 max)


def _build_coeffs():
    """Return [128, 512] fp32 = concat([W0T, W1T, M00T, M10T], axis=1),
    each 128x128 transposed for use as matmul lhsT (lhsT[k, m] = M[m, k])."""
    dt = np.float64
    A = np.array([[1 - ALPHA, 1 - ALPHA],
                  [-ALPHA * BETA, BETA * (1 - ALPHA) + 1 - BETA]], dtype=dt)
    c = np.array([ALPHA, ALPHA * BETA], dtype=dt)
    n = 2 * P
    Apow = np.empty((n + 1, 2, 2), dtype=dt)
    Apow[0] = np.eye(2)
    for j in range(1, n + 1):
        Apow[j] = Apow[j - 1] @ A
    w = np.einsum('jab,b->ja', Apow, c)[:, 0]  # w[j] = (A^j c)[0]

    # Exact coefficient rows for the first two blocks (initial conditions:
    # s0 = x0, b0 = x1 - x0 fold into columns 0 and 1).
    G2 = np.zeros((n, n), dtype=dt)
    G2[0, 0] = 1.0
    for t in range(1, n):
        G2[t, 2:t + 1] = w[t - 2::-1][:max(t - 1, 0)]
        G2[t, 1] = w[t - 1] + Apow[t][0, 1]
        G2[t, 0] = Apow[t][0, 0] - Apow[t][0, 1]

    r = np.arange(P)
    jmat = r[:, None] - r[None, :]
    W0 = np.where(jmat >= 0, w[np.clip(jmat, 0, n)], 0.0)
    W1 = w[P + jmat]
    M00 = G2[0:P, 0:P]
    M10 = G2[P:2 * P, 0:P]
    packed = np.concatenate([W0.T, W1.T, M00.T, M10.T], axis=1)
    return np.ascontiguousarray(packed.astype(np.float32))


def _fix_multi_waits(nc):
    """The walrus build in this container rejects instructions with more than
    one sync wait ("Too many sync wait commands" in setupSyncWait).  Move all
    but the last wait of any multi-wait instruction onto freshly inserted
    same-engine NoOps placed immediately before it (same sequencer, earlier
    program order => semantically equivalent)."""
    import concourse.mybir as mybir

    for f in nc.m.functions:
        for bb in f.blocks:
            insts = bb.instructions
            if not any(
                i.sync_info and i.sync_info.on_wait and len(i.sync_info.on_wait) > 1
                for i in insts
            ):
                continue
            new = []
            for inst in insts:
                si = inst.sync_info
                waits = list(si.on_wait) if si and si.on_wait else []
                if len(waits) > 1:
                    for k, w in enumerate(waits[:-1]):
                        new.append(mybir.InstNoOp(
                            name=f"{inst.name}-wsplit{k}",
                            sync_info=mybir.SyncInfo(on_wait=[w], on_update=[]),
                            bass_nofuse=True,
                            engine=inst.engine,
                        ))
                    si.on_wait = [waits[-1]]
                    inst.sync_info = si
                new.append(inst)
            bb.instructions = new


def build_bass():
    """Build the per-core Bass module (SPMD: same NEFF on all 8 cores)."""
    import concourse.bass as bass
    import concourse.mybir as mybir
    from concourse.tile import TileContext

    f32 = mybir.dt.float32

    nc = bass.Bass()
    x = nc.dram_tensor("x", [BL, T, C], f32, kind="ExternalInput")
    wts = nc.dram_tensor("wts", [P, 4 * P], f32, kind="ExternalInput")
    res = nc.dram_tensor("res", [BL, T, C], f32, kind="ExternalOutput")
    ma = nc.dram_tensor("ma", [BL, T, C], f32, kind="ExternalOutput")

    # DRAM views: [g, p(time-within-block), j(block-in-group), b, c]
    xv = x.rearrange("b (g j p) c -> g p j b c", g=NG, j=G, p=P)
    resv = res.rearrange("b (g j p) c -> g p j b c", g=NG, j=G, p=P)
    mav = ma.rearrange("b (g j p) c -> g p j b c", g=NG, j=G, p=P)

    with TileContext(nc) as tc:
        with (
            tc.tile_pool(name="wpool", bufs=1) as wpool,
            tc.tile_pool(name="xpool", bufs=NG) as xpool,
            tc.tile_pool(name="mapool", bufs=2) as mapool,
            tc.tile_pool(name="respool", bufs=2) as respool,
            tc.tile_pool(name="psum", bufs=6, space="PSUM") as psumpool,
        ):
            wt = wpool.tile([P, 4 * P], f32)
            nc.sync.dma_start(out=wt[:], in_=wts[:])
            w0t = wt[:, 0 * P:1 * P]
            w1t = wt[:, 1 * P:2 * P]
            m00t = wt[:, 2 * P:3 * P]
            m10t = wt[:, 3 * P:4 * P]

            def r4(tile):
                return tile[:].rearrange("p (j b c) -> p j b c", j=G, b=BL, c=C)

            xgs = []
            for g in range(NG):
                xg = xpool.tile([P, G * FREE], f32)
                # DMA APs are limited to 3 dims after balancing -> one DMA
                # per local batch (512 KB each).
                for b in range(BL):
                    nc.sync.dma_start(out=r4(xg)[:, :, b, :], in_=xv[g][:, :, b, :])
                xgs.append(xg)
                mag = mapool.tile([P, G * FREE], f32)
                resg = respool.tile([P, G * FREE], f32)
                for j in range(G):
                    i = g * G + j  # global block index
                    xc = xg[:, j * FREE:(j + 1) * FREE]
                    if j > 0:
                        xp = xg[:, (j - 1) * FREE:j * FREE]
                    elif g > 0:
                        xp = xgs[g - 1][:, (G - 1) * FREE:G * FREE]
                    else:
                        xp = None
                    ps = psumpool.tile([P, FREE], f32)
                    if i == 0:
                        nc.tensor.matmul(ps[:], m00t, xc, start=True, stop=True)
                    else:
                        lhs1 = m10t if i == 1 else w1t
                        nc.tensor.matmul(ps[:], w0t, xc, start=True, stop=False)
                        nc.tensor.matmul(ps[:], lhs1, xp, start=False, stop=True)
                    ma_sec = mag[:, j * FREE:(j + 1) * FREE]
                    res_sec = resg[:, j * FREE:(j + 1) * FREE]
                    nc.scalar.copy(out=ma_sec, in_=ps[:])
                    nc.vector.tensor_sub(out=res_sec, in0=xc, in1=ps[:])
                # Spread stores across the other DMA paths so the SP ring
                # only carries loads: ma on the ACT HWDGE ring, res on the
                # gpsimd SWDGE ring.
                for b in range(BL):
                    nc.scalar.dma_start(out=mav[g][:, :, b, :], in_=r4(mag)[:, :, b, :])
                    nc.gpsimd.dma_start(out=resv[g][:, :, b, :], in_=r4(resg)[:, :, b, :])
    _fix_multi_waits(nc)
    return nc


_CACHE = {}


def kernel(x):
    from concourse.bass_utils import run_bass_kernel_spmd

    x = np.ascontiguousarray(np.asarray(x), dtype=np.float32)
    assert x.shape == (B, T, C), x.shape

    if "nc" not in _CACHE:
        _CACHE["nc"] = build_bass()
        _CACHE["wts"] = _build_coeffs()
    nc = _CACHE["nc"]
    wts = _CACHE["wts"]

    in_maps = [
        {"x": np.ascontiguousarray(x[i * BL:(i + 1) * BL]), "wts": wts}
        for i in range(NCORES)
    ]
    r = run_bass_kernel_spmd(nc, in_maps, core_ids=list(range(NCORES)))
    res = np.concatenate([r.results[i]["res"] for i in range(NCORES)], axis=0)
    ma = np.concatenate([r.results[i]["ma"] for i in range(NCORES)], axis=0)
    return res, ma


# revision 9
# speedup vs baseline: 1.2605x; 1.2605x over previous
"""Trainium2 Bass kernel for DEMA (Holt's linear trend) decomposition.

reference:  ma = DEMA(x) along time (alpha=0.3, beta=0.1), res = x - ma,
            x: [32, 4096, 128] fp32, returns (res, ma).

Approach: the DEMA is a 2x2 linear recurrence v_t = A v_{t-1} + c x_t with
spectral radius sqrt(0.7) ~ 0.837, so the impulse response decays below 1e-10
within 128 steps.  The scan therefore collapses to a banded lower-triangular
matmul (FIR) over time:  with 128-step time blocks,
    ma_blk[i] = W0 @ x_blk[i] + W1 @ x_blk[i-1]
with constant 128x128 Toeplitz coefficient blocks (W0 lower-triangular band,
W1 the band crossing the block boundary).  Blocks 0/1 get exact special
matrices carrying the s0/b0 initial-condition terms.  This maps onto the
TensorEngine: contraction over source-time (partitions), (batch x channel)
on the moving free dim.

Sharding: batch 32 -> 4 per core across 8 cores, no communication.
"""

import numpy as np

ALPHA = 0.3
BETA = 0.1
P = 128          # time block = partition dim
B, T, C = 32, 4096, 128
NCORES = 8
BL = B // NCORES  # local batch = 4
NB = T // P       # 32 time blocks
G = 8             # time blocks per DMA group
NG = NB // G      # groups
FREE = BL * C     # matmul moving free dim = 512 (fp32 max)


def _build_coeffs():
    """Return [128, 512] fp32 = concat([W0T, W1T, M00T, M10T], axis=1),
    each 128x128 transposed for use as matmul lhsT (lhsT[k, m] = M[m, k])."""
    dt = np.float64
    A = np.array([[1 - ALPHA, 1 - ALPHA],
                  [-ALPHA * BETA, BETA * (1 - ALPHA) + 1 - BETA]], dtype=dt)
    c = np.array([ALPHA, ALPHA * BETA], dtype=dt)
    n = 2 * P
    Apow = np.empty((n + 1, 2, 2), dtype=dt)
    Apow[0] = np.eye(2)
    for j in range(1, n + 1):
        Apow[j] = Apow[j - 1] @ A
    w = np.einsum('jab,b->ja', Apow, c)[:, 0]  # w[j] = (A^j c)[0]

    # Exact coefficient rows for the first two blocks (initial conditions:
    # s0 = x0, b0 = x1 - x0 fold into columns 0 and 1).
    G2 = np.zeros((n, n), dtype=dt)
    G2[0, 0] = 1.0
    for t in range(1, n):
        G2[t, 2:t + 1] = w[t - 2::-1][:max(t - 1, 0)]
        G2[t, 1] = w[t - 1] + Apow[t][0, 1]
        G2[t, 0] = Apow[t][0, 0] - Apow[t][0, 1]

    r = np.arange(P)
    jmat = r[:, None] - r[None, :]
    W0 = np.where(jmat >= 0, w[np.clip(jmat, 0, n)], 0.0)
    W1 = w[P + jmat]
    M00 = G2[0:P, 0:P]
    M10 = G2[P:2 * P, 0:P]
    packed = np.concatenate([W0.T, W1.T, M00.T, M10.T], axis=1)
    return np.ascontiguousarray(packed.astype(np.float32))


def _fix_multi_waits(nc):
    """The walrus build in this container rejects instructions with more than
    one sync wait ("Too many sync wait commands" in setupSyncWait).  Move all
    but the last wait of any multi-wait instruction onto freshly inserted
    same-engine NoOps placed immediately before it (same sequencer, earlier
    program order => semantically equivalent)."""
    import concourse.mybir as mybir

    for f in nc.m.functions:
        for bb in f.blocks:
            insts = bb.instructions
            if not any(
                i.sync_info and i.sync_info.on_wait and len(i.sync_info.on_wait) > 1
                for i in insts
            ):
                continue
            new = []
            for inst in insts:
                si = inst.sync_info
                waits = list(si.on_wait) if si and si.on_wait else []
                if len(waits) > 1:
                    for k, w in enumerate(waits[:-1]):
                        new.append(mybir.InstNoOp(
                            name=f"{inst.name}-wsplit{k}",
                            sync_info=mybir.SyncInfo(on_wait=[w], on_update=[]),
                            bass_nofuse=True,
                            engine=inst.engine,
                        ))
                    si.on_wait = [waits[-1]]
                    inst.sync_info = si
                new.append(inst)
            bb.instructions = new


WARMUP_MM = 5    # dummy matmuls to lift the PE out of the cold HAM state
SC = 4           # blocks per store chunk (store DMA granularity)


def build_bass():
    """Build the per-core Bass module (SPMD: same NEFF on all 8 cores)."""
    import concourse.bass as bass
    import concourse.mybir as mybir
    from concourse.tile import TileContext

    f32 = mybir.dt.float32

    nc = bass.Bass()
    x = nc.dram_tensor("x", [BL, T, C], f32, kind="ExternalInput")
    wts = nc.dram_tensor("wts", [P, 4 * P], f32, kind="ExternalInput")
    res = nc.dram_tensor("res", [BL, T, C], f32, kind="ExternalOutput")
    ma = nc.dram_tensor("ma", [BL, T, C], f32, kind="ExternalOutput")

    # DRAM views: [g, p(time-within-block), j(block-in-group), b, c]
    xv = x.rearrange("b (g j p) c -> g p j b c", g=NG, j=G, p=P)
    # store-chunk views: [s, p, j(block-in-chunk), b, c]
    NS = NB // SC
    resv = res.rearrange("b (s j p) c -> s p j b c", s=NS, j=SC, p=P)
    mav = ma.rearrange("b (s j p) c -> s p j b c", s=NS, j=SC, p=P)

    with TileContext(nc) as tc:
        with (
            tc.tile_pool(name="wpool", bufs=1) as wpool,
            tc.tile_pool(name="xpool", bufs=NG) as xpool,
            tc.tile_pool(name="mapool", bufs=4) as mapool,
            tc.tile_pool(name="respool", bufs=4) as respool,
            tc.tile_pool(name="psum", bufs=6, space="PSUM") as psumpool,
            tc.tile_pool(name="warmps", bufs=1, space="PSUM") as warmpool,
        ):
            wt = wpool.tile([P, 4 * P], f32)
            nc.sync.dma_start(out=wt[:], in_=wts[:])
            w0t = wt[:, 0 * P:1 * P]
            w1t = wt[:, 1 * P:2 * P]
            m00t = wt[:, 2 * P:3 * P]
            m10t = wt[:, 3 * P:4 * P]

            # PE warm-up while the first x group loads: the HAM clock gate
            # needs ~3.4us of sustained activity to unthrottle 1.2->2.4 GHz.
            wps = warmpool.tile([P, 4 * P], f32)
            for _ in range(WARMUP_MM):
                nc.tensor.matmul(wps[:], wt[:, 0:P], wt[:], start=True, stop=True)

            def rx(tile):
                return tile[:].rearrange("p (j b c) -> p j b c", j=G, b=BL, c=C)

            def rs(tile):
                return tile[:].rearrange("p (j b c) -> p j b c", j=SC, b=BL, c=C)

            xgs = []
            for g in range(NG):
                xg = xpool.tile([P, G * FREE], f32)
                # DMA APs are limited to 3 dims after balancing -> one DMA
                # per local batch (512 KB each).
                for b in range(BL):
                    nc.sync.dma_start(out=rx(xg)[:, :, b, :], in_=xv[g][:, :, b, :])
                xgs.append(xg)

            store_ring = [nc.scalar, nc.gpsimd, nc.sync]
            ring_i = 0
            mac = resc = None
            for i in range(NB):
                g, j = divmod(i, G)
                xg = xgs[g]
                xc = xg[:, j * FREE:(j + 1) * FREE]
                if j > 0:
                    xp = xg[:, (j - 1) * FREE:j * FREE]
                elif g > 0:
                    xp = xgs[g - 1][:, (G - 1) * FREE:G * FREE]
                else:
                    xp = None
                if i % SC == 0:
                    mac = mapool.tile([P, SC * FREE], f32)
                    resc = respool.tile([P, SC * FREE], f32)
                jc = i % SC
                ps = psumpool.tile([P, FREE], f32)
                if i == 0:
                    nc.tensor.matmul(ps[:], m00t, xc, start=True, stop=True)
                else:
                    lhs1 = m10t if i == 1 else w1t
                    nc.tensor.matmul(ps[:], w0t, xc, start=True, stop=False)
                    nc.tensor.matmul(ps[:], lhs1, xp, start=False, stop=True)
                ma_sec = mac[:, jc * FREE:(jc + 1) * FREE]
                res_sec = resc[:, jc * FREE:(jc + 1) * FREE]
                nc.scalar.copy(out=ma_sec, in_=ps[:])
                nc.vector.tensor_sub(out=res_sec, in0=xc, in1=ps[:])
                if jc == SC - 1:
                    s = i // SC
                    # Rotate stores across the three DMA issue paths (ACT
                    # HWDGE, gpsimd SWDGE, SP HWDGE after loads are done).
                    for b in range(BL):
                        e1 = store_ring[ring_i % 3]; ring_i += 1
                        e2 = store_ring[ring_i % 3]; ring_i += 1
                        e1.dma_start(out=mav[s][:, :, b, :], in_=rs(mac)[:, :, b, :])
                        e2.dma_start(out=resv[s][:, :, b, :], in_=rs(resc)[:, :, b, :])
    _fix_multi_waits(nc)
    return nc


_CACHE = {}


def kernel(x):
    from concourse.bass_utils import run_bass_kernel_spmd

    x = np.ascontiguousarray(np.asarray(x), dtype=np.float32)
    assert x.shape == (B, T, C), x.shape

    if "nc" not in _CACHE:
        _CACHE["nc"] = build_bass()
        _CACHE["wts"] = _build_coeffs()
    nc = _CACHE["nc"]
    wts = _CACHE["wts"]

    in_maps = [
        {"x": np.ascontiguousarray(x[i * BL:(i + 1) * BL]), "wts": wts}
        for i in range(NCORES)
    ]
    r = run_bass_kernel_spmd(nc, in_maps, core_ids=list(range(NCORES)))
    res = np.concatenate([r.results[i]["res"] for i in range(NCORES)], axis=0)
    ma = np.concatenate([r.results[i]["ma"] for i in range(NCORES)], axis=0)
    return res, ma
